# revision 31
# baseline (speedup 1.0000x reference)
"""Trainium2 Bass kernel for nn_NeuralTensorDiagLayer.

Computes out = tanh(concat([e1, e2], -1) @ V + diag + b) where
diag[k] = (sum_b(e1*e2) @ W[k]) / (B*D), broadcast over batch.

Sharding (8 NeuronCores, 2D: 4 batch groups x 2 k_out halves):
  - Core c handles batch rows [1024*(c//2), 1024*(c//2+1)) and k_out
    columns [1024*(c%2), 1024*(c%2+1)).
  - All main-path tensors are cast to bf16 on the host (V from
    uniform(-1,1), x = concat(e1,e2) transposed): rel-err budget is 2e-2
    and bf16 end-to-end measures ~1e-2, while halving HBM traffic and
    keeping the TensorEngine at 1 col/cycle.
  - x^T and V stream into SBUF fully resident via interleaved [128,1024]
    DMAs (2 KiB lines) ordered so contraction tile j (x1_j, x2_j, v_j,
    v_{16+j}) lands early; the main matmul's first PSUM group chases the
    DMA stream and the rest runs from SBUF at full rate.
  - Main matmul: 3 PSUM groups of (3,3,2) k-tiles x 2 batch-half banks.
    Groups 0/1 drain PSUM->stage with DVE/ScalarE copies split per bank;
    group 2 is tanh'ed directly out of PSUM (ScalarE reads PSUM).
  - diag: per-tile fused mul+reduce partials on DVE as x tiles arrive,
    8-core AllReduce of s=[128,16] (each batch row counted twice -> 0.5
    folded into DIAG_SCALE), then a 256-col diag slice as 16 f32r
    matmuls (N=256 -> 1 cycle/row) pinned AFTER main group 1 in the
    TensorE stream (AllReduce is long done by then; pinning avoids the
    baseline's 17us TensorE stall), AllGather over [[0,2,4,6],[1,3,5,7]]
    assembles each k_out half (diag slice index sc = (c%2)*4 + c//2 is
    applied host-side so the device program stays SPMD-identical).
  - tanh+bias on ScalarE with diag as per-partition bias, fp32 out tiles
    DMA'd per k-tile ([k_out, batch] transposed); host reassembles.
"""

import os
import sys

for _p in ("/opt/trn_rl_repo", "/root/.axon_site/_ro/trn_rl_repo"):
    if os.path.isdir(_p) and _p not in sys.path:
        sys.path.append(_p)

import numpy as np

N_CORES = 8
B, D, K_OUT = 4096, 2048, 2048
FEAT = 2 * D
BG, KH = 4, 2                 # batch groups x kout halves
BPC = B // BG                 # 1024 batch rows per core
KHC = K_OUT // KH             # 1024 kout cols per core
KPC = K_OUT // N_CORES        # 256 diag rows per core
FT = FEAT // 128              # 32 feature tiles
DT = D // 128                 # 16 e1-space feature tiles
KTL = KHC // 128              # 8 local kout tiles
KGROUPS = (3, 3, 2)           # kout tile groups (2*g PSUM banks each)
DIAG_SCALE = 1.0 / (B * D)

_CACHE = {}


def _build_nc():
    import concourse.bacc as bacc
    import concourse.tile as tile
    import concourse.mybir as mybir
    from concourse.tile_rust import add_dep_helper

    dt = mybir.dt
    nc = bacc.Bacc("TRN2", target_bir_lowering=False, debug=False,
                   num_devices=N_CORES)

    xt = nc.dram_tensor("xt", [FEAT, BPC], dt.bfloat16, kind="ExternalInput").ap()
    v = nc.dram_tensor("v", [FEAT, KHC], dt.bfloat16, kind="ExternalInput").ap()
    wt = nc.dram_tensor("wt", [128, DT * KHC], dt.bfloat16,
                        kind="ExternalInput").ap()
    bvec = nc.dram_tensor("bvec", [128, KTL], dt.float32, kind="ExternalInput").ap()
    out = nc.dram_tensor("out", [KHC, BPC], dt.bfloat16, kind="ExternalOutput").ap()
    diag_dbg = nc.dram_tensor("diag_dbg", [128, KTL], dt.float32,
                              kind="ExternalOutput").ap()


    core_ids = list(range(N_CORES))
    ag_groups = [[0, 2, 4, 6], [1, 3, 5, 7]]

    with tile.TileContext(nc) as tc:
        with tc.tile_pool(name="xpool", bufs=1) as xpool, \
             tc.tile_pool(name="vpool", bufs=1) as vpool, \
             tc.tile_pool(name="wpool", bufs=2) as wpool, \
             tc.tile_pool(name="spool", bufs=1) as spool, \
             tc.tile_pool(name="scratch", bufs=2) as scratch, \
             tc.tile_pool(name="stage", bufs=1) as stage_pool, \
             tc.tile_pool(name="opool", bufs=2) as opool, \
             tc.tile_pool(name="psum", bufs=7, space="PSUM") as pp, \
             tc.tile_pool(name="dram", bufs=1, space="DRAM") as dram:

            # ---- interleaved resident loads ----
            # All HWDGE DMAs drain ONE FIFO queue in issue order, so issue
            # exactly in the main loop's consumption order: j-step j needs
            # (x tile j, v tile j). Granularity ramps up (singles -> pairs
            # -> quads) so the first matmuls start ~5us earlier while later
            # transfers stay big. The diag path needs e2 tiles (x tiles
            # 16..31) too, but only by ~mid-kernel, which the paired order
            # delivers anyway.
            x_all = xpool.tile([128, FT * BPC], dt.bfloat16)
            v_all = vpool.tile([128, FT * KHC], dt.bfloat16)

            def multi_load(dst_tile, dst_cols, src_t, tile0, n):
                nc.sync.dma_start(
                    dst_tile[:, tile0 * dst_cols:(tile0 + n) * dst_cols]
                    .rearrange("p (j c) -> p j c", j=n),
                    src_t[tile0 * 128:(tile0 + n) * 128, :]
                    .rearrange("(j p) c -> p j c", p=128))

            for t in range(2):                      # singles: j = 0, 1
                multi_load(x_all, BPC, xt, t, 1)
                multi_load(v_all, KHC, v, t, 1)
            for t in range(1, 12):                  # pairs: j = 2..23
                multi_load(x_all, BPC, xt, 2 * t, 2)
                multi_load(v_all, KHC, v, 2 * t, 2)
            for b in range(6, 8):                   # quads: j = 24..31
                multi_load(x_all, BPC, xt, 4 * b, 4)
                multi_load(v_all, KHC, v, 4 * b, 4)
            # diag-path weights: full kout-half W^T, host-prepacked to
            # [128, DT*KHC], streamed through 2 quarter-sized SBUF buffers
            # (first two quarters here, the rest double-buffered inside the
            # diag chain). Needed only mid-kernel.
            QW = 4 * KHC
            wq = [wpool.tile([128, QW], dt.bfloat16, tag="wq", name="wq0"),
                  wpool.tile([128, QW], dt.bfloat16, tag="wq", name="wq1")]
            nc.sync.dma_start(wq[0][:], wt[:, 0:QW])
            nc.sync.dma_start(wq[1][:], wt[:, QW:2 * QW])
            b_sb = spool.tile([128, KTL], dt.float32, name="b_sb")
            nc.sync.dma_start(b_sb[:], bvec[:])

            # ---- diag partials as x-tile pairs arrive: mul on DVE, the
            # ---- batch-sum via ScalarE Copy+accum (keeps DVE light) ----
            s_sb = spool.tile([128, DT], dt.float32)
            trash = scratch.tile([128, BPC], dt.bfloat16, name="trash")
            for j in range(DT):
                prod = scratch.tile([128, BPC], dt.bfloat16, tag="prod",
                                    name=f"prod{j}")
                nc.vector.tensor_mul(
                    prod[:],
                    x_all[:, j * BPC:(j + 1) * BPC],
                    x_all[:, (DT + j) * BPC:(DT + j + 1) * BPC])
                nc.scalar.activation(trash[:], prod[:],
                                     mybir.ActivationFunctionType.Copy,
                                     accum_out=s_sb[:, j:j + 1])

            # ---- share s across the 4 batch groups: AllGather within the
            # kout-column subgroup (which holds each batch group exactly
            # once) + local DVE reduce. (Small-group AllGather measures
            # 8-14us stable; AllReduce measured 40-130us with huge
            # variance.) ----
            NG = N_CORES // 2
            s_in = dram.tile([128, DT], dt.float32)
            s_gat = dram.tile([NG * 128, DT], dt.float32)
            nc.sync.dma_start(s_in[:], s_sb[:])
            nc.gpsimd.collective_compute(
                "AllGather", mybir.AluOpType.bypass,
                replica_groups=ag_groups,
                ins=[s_in.opt()], outs=[s_gat.opt()])
            s_all = spool.tile([128, NG * DT], dt.float32, name="s_all")
            nc.sync.dma_start(
                s_all[:].rearrange("p (c j) -> p c j", c=NG),
                s_gat[:].rearrange("(c p) j -> p c j", p=128))

            # ---- main matmul: out^T = V_half^T @ x^T, bf16 on TensorE ----
            n_staged = KGROUPS[0] + KGROUPS[1]
            n_last = KGROUPS[2]
            stage = stage_pool.tile([128, n_staged * BPC], dt.float32,
                                    name="stage")
            diag_cols = spool.tile([128, KTL], dt.float32, name="diag_cols")
            k0 = 0
            for kg, g in enumerate(KGROUPS):
                last = kg == len(KGROUPS) - 1
                pss = [[pp.tile([128, 512], dt.float32, tag="ps",
                                name=f"ps{kg}_{q}_{b2}")
                        for b2 in range(2)] for q in range(g)]
                for j in range(FT):
                    for q in range(g):
                        for b2 in range(2):
                            mm = nc.tensor.matmul(
                                pss[q][b2][:],
                                v_all[:, j * KHC + (k0 + q) * 128:
                                      j * KHC + (k0 + q + 1) * 128],
                                x_all[:, j * BPC + b2 * 512:
                                      j * BPC + (b2 + 1) * 512],
                                start=(j == 0), stop=(j == FT - 1))
                if not last:
                    # drain PSUM -> stage. Group 1 drains go on ScalarE:
                    # the staged tanhs behind them in the ACT queue have a
                    # TRUE data dep on them (they read stage), so the
                    # scheduler can never hoist an s-gated op ahead of the
                    # drains and stall the PSUM handoff (every s-gated op
                    # sits at the end of its engine queue).
                    for q in range(g):
                        kt = k0 + q
                        for b2 in range(2):
                            dst = stage[:, kt * BPC + b2 * 512:
                                        kt * BPC + (b2 + 1) * 512]
                            if kg == 1 or b2 == 1:
                                nc.scalar.activation(
                                    dst, pss[q][b2][:],
                                    mybir.ActivationFunctionType.Copy)
                            else:
                                nc.vector.tensor_copy(dst, pss[q][b2][:])
                    if kg == 1:
                        # ---- diag: full kout-half [1, KHC] = s @ W_half^T,
                        # entirely OFF the TensorE/GpSimd streams: per-
                        # partition multiply-accumulate + a 7-step partition-
                        # halving tree on DVE (GpSimd C-reduce measures 32us,
                        # and any PE involvement can stall the matmul stream
                        # when the AllReduce runs late). Pinned after group
                        # 1's drains so the scheduler cannot starve the PSUM
                        # handoff. ----
                        for c in range(1, NG):
                            nc.vector.tensor_add(
                                s_all[:, 0:DT], s_all[:, 0:DT],
                                s_all[:, c * DT:(c + 1) * DT])
                        s_r = s_all
                        accs = [spool.tile([128, KHC], dt.bfloat16,
                                           name=f"acc{i}") for i in range(2)]
                        nc.vector.tensor_scalar_mul(
                            accs[0][:], wq[0][:, 0:KHC], s_r[:, 0:1])
                        for jd in range(1, DT):
                            if jd in (4, 8):
                                # double-buffer the next wt quarter
                                nxt = wpool.tile([128, QW], dt.bfloat16,
                                                 tag="wq", name=f"wq{jd//4+1}")
                                nc.sync.dma_start(
                                    nxt[:], wt[:, (jd // 4 + 1) * QW:
                                               (jd // 4 + 2) * QW])
                                wq.append(nxt)
                            nc.vector.scalar_tensor_tensor(
                                accs[jd % 2][:],
                                wq[jd // 4][:, (jd % 4) * KHC:
                                            (jd % 4 + 1) * KHC],
                                s_r[:, jd:jd + 1],
                                accs[(jd + 1) % 2][:],
                                mybir.AluOpType.mult, mybir.AluOpType.add)
                        acc_f = accs[(DT - 1) % 2]
                        # cross-partition sum: 8 DMA-transposes flip the
                        # 128 partials into the free axis, then one DVE
                        # reduce yields diag_cols [128, KTL] directly
                        # (DVE cannot read across partitions; GpSimd
                        # C-reduce costs 32us; PE would stall on late
                        # AllReduces)
                        rbuf = spool.tile([128, KHC], dt.bfloat16,
                                          name="rbuf")
                        for kb in range(KTL):
                            nc.sync.dma_start(
                                rbuf[:, kb * 128:(kb + 1) * 128],
                                acc_f[:, kb * 128:(kb + 1) * 128],
                                transpose=True)
                        nc.vector.tensor_reduce(
                            diag_cols[:],
                            rbuf[:].rearrange("p (kb q) -> p kb q", kb=KTL),
                            mybir.AxisListType.X, mybir.AluOpType.add)
                        nc.vector.tensor_scalar_mul(diag_cols[:],
                                                    diag_cols[:], DIAG_SCALE)
                        nc.vector.tensor_add(diag_cols[:], diag_cols[:],
                                             b_sb[:])
                        nc.sync.dma_start(diag_dbg[:], diag_cols[:])

                        # tanh for all staged tiles (emitted after the
                        # diag_cols writers in program order -- Tile deps
                        # are program-order); overlaps the last group
                        for kt in range(n_staged):
                            ot = opool.tile([128, BPC], dt.bfloat16, tag="ot",
                                            name=f"ot{kt}")
                            nc.scalar.activation(
                                ot[:], stage[:, kt * BPC:(kt + 1) * BPC],
                                mybir.ActivationFunctionType.Tanh,
                                bias=diag_cols[:, kt:kt + 1])
                            nc.sync.dma_start(out[kt * 128:(kt + 1) * 128, :],
                                              ot[:])
                else:
                    # last group: tanh straight out of PSUM (ScalarE)
                    for q in range(g):
                        kt = k0 + q
                        ot2 = opool.tile([128, BPC], dt.bfloat16, tag="ot",
                                         name=f"ot_last_{q}")
                        for b2 in range(2):
                            nc.scalar.activation(
                                ot2[:, b2 * 512:(b2 + 1) * 512],
                                pss[q][b2][:],
                                mybir.ActivationFunctionType.Tanh,
                                bias=diag_cols[:, kt:kt + 1])
                        nc.sync.dma_start(out[kt * 128:(kt + 1) * 128, :],
                                          ot2[:])
                k0 += g

    nc.compile()
    return nc


def _get_nc():
    if "nc" not in _CACHE:
        _CACHE["nc"] = _build_nc()
    return _CACHE["nc"]


def make_in_maps(e1, e2, W, V, b):
    import ml_dtypes
    bf16 = ml_dtypes.bfloat16

    in_maps = []
    for c in range(N_CORES):
        g, h = c // 2, c % 2
        rows = slice(g * BPC, (g + 1) * BPC)
        hcols = slice(h * KHC, (h + 1) * KHC)
        xt = np.ascontiguousarray(
            np.concatenate([e1[rows], e2[rows]], axis=1).T).astype(bf16)
        wt_half = np.ascontiguousarray(
            W[hcols].T.reshape(DT, 128, KHC).transpose(1, 0, 2)
            .reshape(128, DT * KHC)).astype(bf16)
        in_maps.append({
            "xt": xt,
            "v": np.ascontiguousarray(V[:, hcols]).astype(bf16),
            "wt": wt_half,
            "bvec": np.ascontiguousarray(
                b[hcols].reshape(KTL, 128).T),
        })
    return in_maps


def kernel(e1, e2, W, V, b):
    from concourse.bass_utils import run_bass_kernel_spmd

    e1 = np.asarray(e1, dtype=np.float32)
    e2 = np.asarray(e2, dtype=np.float32)
    W = np.asarray(W, dtype=np.float32)
    V = np.asarray(V, dtype=np.float32)
    b = np.asarray(b, dtype=np.float32)

    nc = _get_nc()
    res = run_bass_kernel_spmd(nc, make_in_maps(e1, e2, W, V, b),
                               list(range(N_CORES)))
    _CACHE["last_res"] = res
    out = np.empty((B, K_OUT), dtype=np.float32)
    for c in range(N_CORES):
        g, h = c // 2, c % 2
        out[g * BPC:(g + 1) * BPC, h * KHC:(h + 1) * KHC] = \
            res.results[c]["out"].T.astype(np.float32)
    return out


# revision 32
# speedup vs baseline: 1.0429x; 1.0429x over previous
"""Trainium2 Bass kernel for nn_NeuralTensorDiagLayer.

Computes out = tanh(concat([e1, e2], -1) @ V + diag + b) where
diag[k] = (sum_b(e1*e2) @ W[k]) / (B*D), broadcast over batch.

Sharding (8 NeuronCores, 2D: 4 batch groups x 2 k_out halves):
  - Core c handles batch rows [1024*(c//2), 1024*(c//2+1)) and k_out
    columns [1024*(c%2), 1024*(c%2+1)).
  - All main-path tensors are cast to bf16 on the host (V from
    uniform(-1,1), x = concat(e1,e2) transposed): rel-err budget is 2e-2
    and bf16 end-to-end measures ~1e-2, while halving HBM traffic and
    keeping the TensorEngine at 1 col/cycle.
  - x^T and V stream into SBUF fully resident via interleaved [128,1024]
    DMAs (2 KiB lines) ordered so contraction tile j (x1_j, x2_j, v_j,
    v_{16+j}) lands early; the main matmul's first PSUM group chases the
    DMA stream and the rest runs from SBUF at full rate.
  - Main matmul: 3 PSUM groups of (3,3,2) k-tiles x 2 batch-half banks.
    Groups 0/1 drain PSUM->stage with DVE/ScalarE copies split per bank;
    group 2 is tanh'ed directly out of PSUM (ScalarE reads PSUM).
  - diag: per-tile fused mul+reduce partials on DVE as x tiles arrive,
    8-core AllReduce of s=[128,16] (each batch row counted twice -> 0.5
    folded into DIAG_SCALE), then a 256-col diag slice as 16 f32r
    matmuls (N=256 -> 1 cycle/row) pinned AFTER main group 1 in the
    TensorE stream (AllReduce is long done by then; pinning avoids the
    baseline's 17us TensorE stall), AllGather over [[0,2,4,6],[1,3,5,7]]
    assembles each k_out half (diag slice index sc = (c%2)*4 + c//2 is
    applied host-side so the device program stays SPMD-identical).
  - tanh+bias on ScalarE with diag as per-partition bias, fp32 out tiles
    DMA'd per k-tile ([k_out, batch] transposed); host reassembles.
"""

import os
import sys

for _p in ("/opt/trn_rl_repo", "/root/.axon_site/_ro/trn_rl_repo"):
    if os.path.isdir(_p) and _p not in sys.path:
        sys.path.append(_p)

import numpy as np

N_CORES = 8
B, D, K_OUT = 4096, 2048, 2048
FEAT = 2 * D
BG, KH = 4, 2                 # batch groups x kout halves
BPC = B // BG                 # 1024 batch rows per core
KHC = K_OUT // KH             # 1024 kout cols per core
KPC = K_OUT // N_CORES        # 256 diag rows per core
FT = FEAT // 128              # 32 feature tiles
DT = D // 128                 # 16 e1-space feature tiles
KTL = KHC // 128              # 8 local kout tiles
KGROUPS = (3, 3, 2)           # kout tile groups (2*g PSUM banks each)
DIAG_SCALE = 0.5 / (B * D)    # 0.5: 8-core gather double-counts rows

_CACHE = {}


def _build_nc():
    import concourse.bacc as bacc
    import concourse.tile as tile
    import concourse.mybir as mybir
    from concourse.tile_rust import add_dep_helper

    dt = mybir.dt
    nc = bacc.Bacc("TRN2", target_bir_lowering=False, debug=False,
                   num_devices=N_CORES)

    xt = nc.dram_tensor("xt", [FEAT, BPC], dt.bfloat16, kind="ExternalInput").ap()
    v = nc.dram_tensor("v", [FEAT, KHC], dt.bfloat16, kind="ExternalInput").ap()
    wt = nc.dram_tensor("wt", [128, DT * KHC], dt.bfloat16,
                        kind="ExternalInput").ap()
    bvec = nc.dram_tensor("bvec", [128, KTL], dt.float32, kind="ExternalInput").ap()
    out = nc.dram_tensor("out", [KHC, BPC], dt.bfloat16, kind="ExternalOutput").ap()
    diag_dbg = nc.dram_tensor("diag_dbg", [128, KTL], dt.float32,
                              kind="ExternalOutput").ap()


    core_ids = list(range(N_CORES))
    ag_groups = [[0, 2, 4, 6], [1, 3, 5, 7]]

    with tile.TileContext(nc) as tc:
        with tc.tile_pool(name="xpool", bufs=1) as xpool, \
             tc.tile_pool(name="vpool", bufs=1) as vpool, \
             tc.tile_pool(name="wpool", bufs=2) as wpool, \
             tc.tile_pool(name="spool", bufs=1) as spool, \
             tc.tile_pool(name="scratch", bufs=2) as scratch, \
             tc.tile_pool(name="stage", bufs=1) as stage_pool, \
             tc.tile_pool(name="opool", bufs=2) as opool, \
             tc.tile_pool(name="psum", bufs=7, space="PSUM") as pp, \
             tc.tile_pool(name="dram", bufs=1, space="DRAM") as dram:

            # ---- interleaved resident loads ----
            # All HWDGE DMAs drain ONE FIFO queue in issue order, so issue
            # exactly in the main loop's consumption order: j-step j needs
            # (x tile j, v tile j). Granularity ramps up (singles -> pairs
            # -> quads) so the first matmuls start ~5us earlier while later
            # transfers stay big. The diag path needs e2 tiles (x tiles
            # 16..31) too, but only by ~mid-kernel, which the paired order
            # delivers anyway.
            x_all = xpool.tile([128, FT * BPC], dt.bfloat16)
            v_all = vpool.tile([128, FT * KHC], dt.bfloat16)

            def multi_load(dst_tile, dst_cols, src_t, tile0, n):
                nc.sync.dma_start(
                    dst_tile[:, tile0 * dst_cols:(tile0 + n) * dst_cols]
                    .rearrange("p (j c) -> p j c", j=n),
                    src_t[tile0 * 128:(tile0 + n) * 128, :]
                    .rearrange("(j p) c -> p j c", p=128))

            for t in range(2):                      # singles: j = 0, 1
                multi_load(x_all, BPC, xt, t, 1)
                multi_load(v_all, KHC, v, t, 1)
            for t in range(1, 12):                  # pairs: j = 2..23
                multi_load(x_all, BPC, xt, 2 * t, 2)
                multi_load(v_all, KHC, v, 2 * t, 2)
            for b in range(6, 8):                   # quads: j = 24..31
                multi_load(x_all, BPC, xt, 4 * b, 4)
                multi_load(v_all, KHC, v, 4 * b, 4)
            # diag-path weights: full kout-half W^T, host-prepacked to
            # [128, DT*KHC], streamed through 2 quarter-sized SBUF buffers
            # (first two quarters here, the rest double-buffered inside the
            # diag chain). Needed only mid-kernel.
            QW = 4 * KHC
            wq = [wpool.tile([128, QW], dt.bfloat16, tag="wq", name="wq0"),
                  wpool.tile([128, QW], dt.bfloat16, tag="wq", name="wq1")]
            nc.sync.dma_start(wq[0][:], wt[:, 0:QW])
            nc.sync.dma_start(wq[1][:], wt[:, QW:2 * QW])
            b_sb = spool.tile([128, KTL], dt.float32, name="b_sb")
            nc.sync.dma_start(b_sb[:], bvec[:])

            # ---- diag partials as x-tile pairs arrive: mul on DVE, the
            # ---- batch-sum via ScalarE Copy+accum (keeps DVE light) ----
            s_sb = spool.tile([128, DT], dt.float32)
            trash = scratch.tile([128, BPC], dt.bfloat16, name="trash")
            for j in range(DT):
                prod = scratch.tile([128, BPC], dt.bfloat16, tag="prod",
                                    name=f"prod{j}")
                nc.vector.tensor_mul(
                    prod[:],
                    x_all[:, j * BPC:(j + 1) * BPC],
                    x_all[:, (DT + j) * BPC:(DT + j + 1) * BPC])
                nc.scalar.activation(trash[:], prod[:],
                                     mybir.ActivationFunctionType.Copy,
                                     accum_out=s_sb[:, j:j + 1])

            # ---- share s across the 4 batch groups: AllGather within the
            # kout-column subgroup (which holds each batch group exactly
            # once) + local DVE reduce. (Small-group AllGather measures
            # 8-14us stable; AllReduce measured 40-130us with huge
            # variance.) ----
            NG = N_CORES
            s_in = dram.tile([128, DT], dt.float32)
            s_gat = dram.tile([NG * 128, DT], dt.float32,
                              addr_space="Shared")
            nc.sync.dma_start(s_in[:], s_sb[:])
            nc.gpsimd.collective_compute(
                "AllGather", mybir.AluOpType.bypass,
                replica_groups=[core_ids],
                ins=[s_in.opt()], outs=[s_gat.opt()])
            s_all = spool.tile([128, NG * DT], dt.float32, name="s_all")
            nc.sync.dma_start(
                s_all[:].rearrange("p (c j) -> p c j", c=NG),
                s_gat[:].rearrange("(c p) j -> p c j", p=128))

            # ---- main matmul: out^T = V_half^T @ x^T, bf16 on TensorE ----
            n_staged = KGROUPS[0] + KGROUPS[1]
            n_last = KGROUPS[2]
            stage = stage_pool.tile([128, n_staged * BPC], dt.float32,
                                    name="stage")
            diag_cols = spool.tile([128, KTL], dt.float32, name="diag_cols")
            k0 = 0
            for kg, g in enumerate(KGROUPS):
                last = kg == len(KGROUPS) - 1
                pss = [[pp.tile([128, 512], dt.float32, tag="ps",
                                name=f"ps{kg}_{q}_{b2}")
                        for b2 in range(2)] for q in range(g)]
                for j in range(FT):
                    for q in range(g):
                        for b2 in range(2):
                            mm = nc.tensor.matmul(
                                pss[q][b2][:],
                                v_all[:, j * KHC + (k0 + q) * 128:
                                      j * KHC + (k0 + q + 1) * 128],
                                x_all[:, j * BPC + b2 * 512:
                                      j * BPC + (b2 + 1) * 512],
                                start=(j == 0), stop=(j == FT - 1))
                if not last:
                    # drain PSUM -> stage. Group 1 drains go on ScalarE:
                    # the staged tanhs behind them in the ACT queue have a
                    # TRUE data dep on them (they read stage), so the
                    # scheduler can never hoist an s-gated op ahead of the
                    # drains and stall the PSUM handoff (every s-gated op
                    # sits at the end of its engine queue).
                    for q in range(g):
                        kt = k0 + q
                        for b2 in range(2):
                            dst = stage[:, kt * BPC + b2 * 512:
                                        kt * BPC + (b2 + 1) * 512]
                            if kg == 1 or b2 == 1:
                                nc.scalar.activation(
                                    dst, pss[q][b2][:],
                                    mybir.ActivationFunctionType.Copy)
                            else:
                                nc.vector.tensor_copy(dst, pss[q][b2][:])
                    if kg == 1:
                        # ---- diag: full kout-half [1, KHC] = s @ W_half^T,
                        # entirely OFF the TensorE/GpSimd streams: per-
                        # partition multiply-accumulate + a 7-step partition-
                        # halving tree on DVE (GpSimd C-reduce measures 32us,
                        # and any PE involvement can stall the matmul stream
                        # when the AllReduce runs late). Pinned after group
                        # 1's drains so the scheduler cannot starve the PSUM
                        # handoff. ----
                        for c in range(1, NG):
                            nc.vector.tensor_add(
                                s_all[:, 0:DT], s_all[:, 0:DT],
                                s_all[:, c * DT:(c + 1) * DT])
                        s_r = s_all
                        accs = [spool.tile([128, KHC], dt.bfloat16,
                                           name=f"acc{i}") for i in range(2)]
                        nc.vector.tensor_scalar_mul(
                            accs[0][:], wq[0][:, 0:KHC], s_r[:, 0:1])
                        for jd in range(1, DT):
                            if jd in (4, 8):
                                # double-buffer the next wt quarter
                                nxt = wpool.tile([128, QW], dt.bfloat16,
                                                 tag="wq", name=f"wq{jd//4+1}")
                                nc.sync.dma_start(
                                    nxt[:], wt[:, (jd // 4 + 1) * QW:
                                               (jd // 4 + 2) * QW])
                                wq.append(nxt)
                            nc.vector.scalar_tensor_tensor(
                                accs[jd % 2][:],
                                wq[jd // 4][:, (jd % 4) * KHC:
                                            (jd % 4 + 1) * KHC],
                                s_r[:, jd:jd + 1],
                                accs[(jd + 1) % 2][:],
                                mybir.AluOpType.mult, mybir.AluOpType.add)
                        acc_f = accs[(DT - 1) % 2]
                        # cross-partition sum: 8 DMA-transposes flip the
                        # 128 partials into the free axis, then one DVE
                        # reduce yields diag_cols [128, KTL] directly
                        # (DVE cannot read across partitions; GpSimd
                        # C-reduce costs 32us; PE would stall on late
                        # AllReduces)
                        rbuf = spool.tile([128, KHC], dt.bfloat16,
                                          name="rbuf")
                        for kb in range(KTL):
                            # split across the two HWDGE queues (sync +
                            # scalar) so the 8 transposes run ~2x faster
                            eng = nc.sync if kb % 2 == 0 else nc.scalar
                            eng.dma_start(
                                rbuf[:, kb * 128:(kb + 1) * 128],
                                acc_f[:, kb * 128:(kb + 1) * 128],
                                transpose=True)
                        nc.vector.tensor_reduce(
                            diag_cols[:],
                            rbuf[:].rearrange("p (kb q) -> p kb q", kb=KTL),
                            mybir.AxisListType.X, mybir.AluOpType.add)
                        nc.vector.tensor_scalar_mul(diag_cols[:],
                                                    diag_cols[:], DIAG_SCALE)
                        nc.vector.tensor_add(diag_cols[:], diag_cols[:],
                                             b_sb[:])
                        nc.sync.dma_start(diag_dbg[:], diag_cols[:])

                        # tanh for all staged tiles (emitted after the
                        # diag_cols writers in program order -- Tile deps
                        # are program-order); overlaps the last group
                        for kt in range(n_staged):
                            ot = opool.tile([128, BPC], dt.bfloat16, tag="ot",
                                            name=f"ot{kt}")
                            nc.scalar.activation(
                                ot[:], stage[:, kt * BPC:(kt + 1) * BPC],
                                mybir.ActivationFunctionType.Tanh,
                                bias=diag_cols[:, kt:kt + 1])
                            nc.sync.dma_start(out[kt * 128:(kt + 1) * 128, :],
                                              ot[:])
                else:
                    # last group: tanh straight out of PSUM (ScalarE)
                    for q in range(g):
                        kt = k0 + q
                        ot2 = opool.tile([128, BPC], dt.bfloat16, tag="ot",
                                         name=f"ot_last_{q}")
                        for b2 in range(2):
                            nc.scalar.activation(
                                ot2[:, b2 * 512:(b2 + 1) * 512],
                                pss[q][b2][:],
                                mybir.ActivationFunctionType.Tanh,
                                bias=diag_cols[:, kt:kt + 1])
                        nc.sync.dma_start(out[kt * 128:(kt + 1) * 128, :],
                                          ot2[:])
                k0 += g

    nc.compile()
    return nc


def _get_nc():
    if "nc" not in _CACHE:
        _CACHE["nc"] = _build_nc()
    return _CACHE["nc"]


def make_in_maps(e1, e2, W, V, b):
    import ml_dtypes
    bf16 = ml_dtypes.bfloat16

    in_maps = []
    for c in range(N_CORES):
        g, h = c // 2, c % 2
        rows = slice(g * BPC, (g + 1) * BPC)
        hcols = slice(h * KHC, (h + 1) * KHC)
        xt = np.ascontiguousarray(
            np.concatenate([e1[rows], e2[rows]], axis=1).T).astype(bf16)
        wt_half = np.ascontiguousarray(
            W[hcols].T.reshape(DT, 128, KHC).transpose(1, 0, 2)
            .reshape(128, DT * KHC)).astype(bf16)
        in_maps.append({
            "xt": xt,
            "v": np.ascontiguousarray(V[:, hcols]).astype(bf16),
            "wt": wt_half,
            "bvec": np.ascontiguousarray(
                b[hcols].reshape(KTL, 128).T),
        })
    return in_maps


def kernel(e1, e2, W, V, b):
    from concourse.bass_utils import run_bass_kernel_spmd

    e1 = np.asarray(e1, dtype=np.float32)
    e2 = np.asarray(e2, dtype=np.float32)
    W = np.asarray(W, dtype=np.float32)
    V = np.asarray(V, dtype=np.float32)
    b = np.asarray(b, dtype=np.float32)

    nc = _get_nc()
    res = run_bass_kernel_spmd(nc, make_in_maps(e1, e2, W, V, b),
                               list(range(N_CORES)))
    _CACHE["last_res"] = res
    out = np.empty((B, K_OUT), dtype=np.float32)
    for c in range(N_CORES):
        g, h = c // 2, c % 2
        out[g * BPC:(g + 1) * BPC, h * KHC:(h + 1) * KHC] = \
            res.results[c]["out"].T.astype(np.float32)
    return out
